# revision 12
# baseline (speedup 1.0000x reference)
"""GRANMixtureBernoulli loss on 8 Trainium2 NeuronCores.

Sharding (per the hint): edges are partitioned by segment; segments are
partitioned across the 8 cores (512 each), so all segment sums stay local
and only the final scalar is combined on the host.

Host prep (data movement only): sort edges by subgraph_idx, place each
segment's edges into a fixed slot range of length L (padded; pad value
contributes exactly 0 to every sum), fold the 0/1 label into the sign of
log_theta (softplus(x) - x*y == softplus((1-2y)*x) for y in {0,1}), and
lay out per-core arrays [128 partitions, NT rows, L slots] in bf16, where
row t = e*K + k of partition p holds segment s = e*128 + p, mixture k.

Device (per core, explicit Bass engine programs; instruction count is the
scarce resource -- per-instruction overhead is ~2-5us and sem round trips
~11us on this stack):
  gpsimd: issues all bulk DMAs (fastest DGE path measured) and pool-avg
          reduces softplus rows -> red_adj/L
  ACT:    full-width Exp then Ln(.+1) per chunk (softplus; exp+ln share
          one table set), software-pipelined, no accum_out
  DVE:    alpha row-sum reduces + batched epilogue (log_softmax,
          logsumexp over K per 128-segment block, 4 ACT round trips)
Host: loss = -(sum of the 8 partial vectors) / E.

Falls back to a pure-numpy evaluation if the device path is unavailable
or shapes are out of the supported envelope.
"""

from contextlib import ExitStack

import numpy as np

E = 4194304
K = 20
S = 4096
N_CORES = 8
SC = S // N_CORES        # segments per core
NE = SC // 128           # 128-segment blocks per core
NT = NE * K              # rows per partition
PAD_THETA = -30.0        # softplus(-30) rounds to exactly 0 through exp+ln
IN_DT = "float16"

_CACHE = {}
LAST_EXEC_NS = None
LAST_RESULTS = None


# ---------------------------------------------------------------- device ----

def _build_nc(L, in_dt_name=IN_DT, G=16, NB_T=3, NB_A=2, repeat=1,
              internal_inputs=False, detect_races=True):
    from concourse import bass, mybir

    dt = mybir.dt
    in_dt = getattr(dt, in_dt_name)
    assert NT % G == 0
    NG = NT // G

    AF = mybir.ActivationFunctionType
    ALU = mybir.AluOpType
    AX = mybir.AxisListType

    nc = bass.Bass(detect_race_conditions=detect_races)
    if internal_inputs:
        # timing variant: identical instruction stream, but bulk inputs live
        # in device DRAM (uninitialized) so dispatch carries no host transfer
        theta = nc.dram_tensor("theta_i", [128, NT * L], in_dt)
        alpha = nc.dram_tensor("alpha_i", [128, NT * L], in_dt)
    else:
        theta = nc.declare_dram_parameter("theta", [128, NT * L], in_dt,
                                          isOutput=False)
        alpha = nc.declare_dram_parameter("alpha", [128, NT * L], in_dt,
                                          isOutput=False)
    rcnt_d = nc.declare_dram_parameter("rcnt", [128, NT], dt.float32,
                                       isOutput=False)
    out_d = nc.declare_dram_parameter("partial", [128, 1], dt.float32,
                                      isOutput=True)

    ctx = ExitStack()
    with ctx:
        block = ctx.enter_context(nc.Block())
        s_th = [ctx.enter_context(nc.semaphore(f"s_th{i}")) for i in range(NB_T)]
        s_al = [ctx.enter_context(nc.semaphore(f"s_al{i}")) for i in range(NB_A)]
        s_rc = ctx.enter_context(nc.semaphore("s_rc"))    # rcnt DMA done
        s_ln = ctx.enter_context(nc.semaphore("s_ln"))    # th slot softplus'd
        s_spr = ctx.enter_context(nc.semaphore("s_spr"))  # th slot reduced
        s_alr = ctx.enter_context(nc.semaphore("s_alr"))  # al slot reduced
        s_d2a = ctx.enter_context(nc.semaphore("s_d2a"))  # epilogue DVE->ACT
        s_a2d = ctx.enter_context(nc.semaphore("s_a2d"))  # epilogue ACT->DVE
        s_out = ctx.enter_context(nc.semaphore("s_out"))  # partial ready
        s_od = ctx.enter_context(nc.semaphore("s_od"))    # out DMA done
        s_init = ctx.enter_context(nc.semaphore("s_init"))  # const memsets done

        th_s = [ctx.enter_context(nc.sbuf_tensor(f"th{i}", [128, G * L], in_dt))
                for i in range(NB_T)]
        al_s = [ctx.enter_context(nc.sbuf_tensor(f"al{i}", [128, G * L], in_dt))
                for i in range(NB_A)]
        red_aj = ctx.enter_context(nc.sbuf_tensor("red_aj", [128, NT], dt.float32))
        red_la = ctx.enter_context(nc.sbuf_tensor("red_la", [128, NT], dt.float32))
        rcnt = ctx.enter_context(nc.sbuf_tensor("rcnt_sb", [128, NT], dt.float32))
        ones = ctx.enter_context(nc.sbuf_tensor("ones", [128, 1], dt.float32))
        zeros = ctx.enter_context(nc.sbuf_tensor("zeros", [128, 1], dt.float32))
        ep_x = ctx.enter_context(nc.sbuf_tensor("ep_x", [128, NT], dt.float32))
        ep_y = ctx.enter_context(nc.sbuf_tensor("ep_y", [128, NT], dt.float32))
        ep_w = ctx.enter_context(nc.sbuf_tensor("ep_w", [128, NT], dt.float32))
        ep_nm = ctx.enter_context(nc.sbuf_tensor("ep_nm", [128, NE], dt.float32))
        ep_s = ctx.enter_context(nc.sbuf_tensor("ep_s", [128, NE], dt.float32))
        ep_sz = ctx.enter_context(nc.sbuf_tensor("ep_sz", [128, NE], dt.float32))
        ep_l = ctx.enter_context(nc.sbuf_tensor("ep_l", [128, NE], dt.float32))
        ep_lz = ctx.enter_context(nc.sbuf_tensor("ep_lz", [128, NE], dt.float32))
        ep_lp = ctx.enter_context(nc.sbuf_tensor("ep_lp", [128, NE], dt.float32))
        part = ctx.enter_context(nc.sbuf_tensor("part", [128, 1], dt.float32))

        ones_ap = ones.ap()[:, 0:1]
        zeros_ap = zeros.ap()[:, 0:1]

        @block.sync
        def _(sp_e: bass.BassEngine):
            sp_e.dma_start(out=rcnt[:], in_=rcnt_d[:]).then_inc(s_rc, 16)
            sp_e.wait_ge(s_out, repeat)
            sp_e.dma_start(out=out_d[:], in_=part[:]).then_inc(s_od, 16)
            sp_e.wait_ge(s_od, 16)

        @block.gpsimd
        def _(gp: bass.BassGpSimd):
            for rep in range(repeat):
                for g in range(NG):
                    gi = rep * NG + g
                    sl = slice(g * G * L, (g + 1) * G * L)
                    it = gi % NB_T
                    ia = gi % NB_A
                    if gi >= NB_T:
                        # th slot free once DVE reduced its softplus values
                        gp.wait_ge(s_spr, gi - NB_T + 1)
                    gp.dma_start(out=th_s[it][:], in_=theta[:, sl]).then_inc(
                        s_th[it], 16)
                    if gi >= NB_A:
                        gp.wait_ge(s_alr, gi - NB_A + 1)
                    gp.dma_start(out=al_s[ia][:], in_=alpha[:, sl]).then_inc(
                        s_al[ia], 16)

        @block.scalar
        def _(act: bass.BassScalarEngine):
            # main loop: in-place softplus on the theta slot (bf16 in+out
            # keeps ACT in its 2x mode); back-to-back same-engine RAW is
            # safe (full-width instructions exceed the pipeline depth)
            act.wait_ge(s_init, 1)
            for rep in range(repeat):
                for g in range(NG):
                    gi = rep * NG + g
                    th = th_s[gi % NB_T]
                    act.wait_ge(s_th[gi % NB_T], 16 * (gi // NB_T + 1))
                    act.activation(out=th[:], in_=th[:], func=AF.Exp,
                                   bias=zeros_ap)
                    act.activation(out=th[:], in_=th[:], func=AF.Ln,
                                   bias=ones_ap).then_inc(s_ln, 1)
                # epilogue: one DVE handoff; accum_out does the K-sums
                act.wait_ge(s_d2a, rep + 1)
                for e in range(NE):
                    act.activation(out=ep_w[:, e * K:(e + 1) * K],
                                   in_=ep_x[:, e * K:(e + 1) * K],
                                   func=AF.Exp, bias=zeros_ap,
                                   accum_out=ep_s[:, e:e + 1])
                    act.activation(out=ep_w[:, e * K:(e + 1) * K],
                                   in_=ep_y[:, e * K:(e + 1) * K],
                                   func=AF.Exp, bias=ep_nm[:, e:e + 1],
                                   accum_out=ep_sz[:, e:e + 1])
                act.activation(out=ep_l[:], in_=ep_s[:], func=AF.Ln,
                               bias=zeros_ap)
                act.activation(out=ep_lz[:], in_=ep_sz[:], func=AF.Ln,
                               bias=zeros_ap).then_inc(s_a2d, 1)

        @block.vector
        def _(ve: bass.BassVectorEngine):
            ve.memset(ones[:, :], 1.0)
            ve.memset(zeros[:, :], 0.0).then_inc(s_init, 1)
            ve.wait_ge(s_rc, 16)
            for rep in range(repeat):
                for g in range(NG):
                    gi = rep * NG + g
                    ia = gi % NB_A
                    ve.wait_ge(s_al[ia], 16 * (gi // NB_A + 1))
                    ve.tensor_reduce(
                        out=red_la[:, g * G:(g + 1) * G],
                        in_=al_s[ia][:, :].rearrange("p (g l) -> p g l", g=G),
                        axis=AX.X,
                        op=ALU.add,
                    ).then_inc(s_alr, 1)
                    ve.wait_ge(s_ln, gi + 1)
                    ve.tensor_reduce(
                        out=red_aj[:, g * G:(g + 1) * G],
                        in_=th_s[gi % NB_T][:, :].rearrange(
                            "p (g l) -> p g l", g=G),
                        axis=AX.X,
                        op=ALU.add,
                    ).then_inc(s_spr, 1)
                # ----- epilogue -----
                # x = red_la/cnt (|x| small: no max-shift needed for its lse)
                # y = x - red_adj;  nm = -max_K(y)
                # log_prob = -nm - lse + ln(sum_K exp(y + nm))
                ve.tensor_tensor(out=ep_x[:], in0=red_la[:], in1=rcnt[:],
                                 op=ALU.mult)
                ve.tensor_tensor(out=ep_y[:], in0=ep_x[:], in1=red_aj[:],
                                 op=ALU.subtract)
                ve.tensor_reduce(out=ep_nm[:],
                                 in_=ep_y[:, :].rearrange("p (e k) -> p e k",
                                                          e=NE),
                                 axis=AX.X, op=ALU.max,
                                 negate=True).then_inc(s_d2a, 1)
                # ACT: ep_s = sum exp(x); ep_sz = sum exp(y + nm);
                #      ep_l = ln(ep_s); ep_lz = ln(ep_sz)
                ve.wait_ge(s_a2d, rep + 1)
                ve.tensor_tensor(out=ep_lp[:], in0=ep_lz[:], in1=ep_nm[:],
                                 op=ALU.subtract)
                ve.tensor_tensor(out=ep_lp[:], in0=ep_lp[:], in1=ep_l[:],
                                 op=ALU.subtract)
                ve.tensor_reduce(out=part[:], in_=ep_lp[:, :], axis=AX.X,
                                 op=ALU.add).then_inc(s_out, 1)

    return nc


# ------------------------------------------------------------- host paths ---

def _prep_inputs(label, log_theta, log_alpha, idx):
    """Sort/pad/fold on the host; returns per-core arrays + L."""
    cnt = np.bincount(idx, minlength=S)
    L = max(128, -(-int(cnt.max()) // 16) * 16)

    order = np.argsort(idx, kind="stable")
    seg_sorted = idx[order]
    starts = np.zeros(S, np.int64)
    np.cumsum(cnt[:-1], out=starts[1:])
    pos = np.arange(E, dtype=np.int64) - starts[seg_sorted]
    dest = seg_sorted * L + pos
    del seg_sorted, starts, pos

    sign = (1.0 - 2.0 * label.astype(np.float32))

    f16 = np.float16
    out = {"L": L, "theta": [], "alpha": [], "rcnt": []}

    pth = np.full((S * L, K), PAD_THETA, np.float32)
    tmp = log_theta[order].astype(np.float32)
    tmp *= sign[order, None]
    pth[dest] = tmp
    del tmp
    # [S, L, K] -> [S, K, L] -> per core [128, NT*L]
    t1 = np.ascontiguousarray(pth.reshape(S, L, K).transpose(0, 2, 1))
    del pth
    for c in range(N_CORES):
        blk = t1[c * SC:(c + 1) * SC].reshape(NE, 128, K, L)
        out["theta"].append(np.ascontiguousarray(
            blk.transpose(1, 0, 2, 3).reshape(128, NT * L).astype(f16)))
    del t1

    pal = np.zeros((S * L, K), np.float32)
    pal[dest] = log_alpha[order].astype(np.float32)
    del order, dest
    a1 = np.ascontiguousarray(pal.reshape(S, L, K).transpose(0, 2, 1))
    del pal
    for c in range(N_CORES):
        blk = a1[c * SC:(c + 1) * SC].reshape(NE, 128, K, L)
        out["alpha"].append(np.ascontiguousarray(
            blk.transpose(1, 0, 2, 3).reshape(128, NT * L).astype(f16)))
    del a1

    with np.errstate(divide="ignore"):
        rcnt = (1.0 / cnt.astype(np.float32))
    for c in range(N_CORES):
        # expanded over k: [128, NT], row t=e*K+k of partition p = segment
        # e*128+p
        rc = rcnt[c * SC:(c + 1) * SC].reshape(NE, 128)
        out["rcnt"].append(np.ascontiguousarray(
            np.repeat(rc[:, :, None], K, axis=2).transpose(1, 0, 2)
            .reshape(128, NT).astype(np.float32)))
    return out


def _np_fallback(label, log_theta, log_alpha, idx):
    adj = np.logaddexp(np.float32(0.0), log_theta) - log_theta * label[:, None]
    const = np.bincount(idx, minlength=S).astype(np.float32)
    red_adj = np.empty((S, K), np.float32)
    red_la = np.empty((S, K), np.float32)
    for k in range(K):
        red_adj[:, k] = np.bincount(idx, weights=adj[:, k], minlength=S)
        red_la[:, k] = np.bincount(idx, weights=log_alpha[:, k], minlength=S)
    rla = red_la / const[:, None]
    m = rla.max(axis=1, keepdims=True)
    rla = rla - (m + np.log(np.exp(rla - m).sum(axis=1, keepdims=True)))
    z = -red_adj + rla
    zm = z.max(axis=1, keepdims=True)
    log_prob = (zm + np.log(np.exp(z - zm).sum(axis=1, keepdims=True)))[:, 0]
    return np.float32(-log_prob.sum(dtype=np.float64) / float(E))


# ------------------------------------------------------------------- entry --

def kernel(label, log_theta, log_alpha, subgraph_idx, _trace=False):
    global LAST_EXEC_NS, LAST_RESULTS
    label = np.asarray(label, np.float32)
    log_theta = np.asarray(log_theta, np.float32)
    log_alpha = np.asarray(log_alpha, np.float32)
    idx = np.asarray(subgraph_idx).astype(np.int64)
    assert label.shape == (E,) and log_theta.shape == (E, K)

    try:
        prep = _prep_inputs(label, log_theta, log_alpha, idx)
        L = prep["L"]
        if L > 4096:
            raise RuntimeError(f"L={L} out of envelope")
        key = ("nc", L, IN_DT)
        if key not in _CACHE:
            _CACHE[key] = _build_nc(L)
        nc = _CACHE[key]
        from concourse.bass_utils import run_bass_kernel_spmd
        in_maps = [{"theta": prep["theta"][c], "alpha": prep["alpha"][c],
                    "rcnt": prep["rcnt"][c]} for c in range(N_CORES)]
        r = run_bass_kernel_spmd(nc, in_maps, list(range(N_CORES)),
                                 trace=_trace)
        LAST_EXEC_NS = r.exec_time_ns
        LAST_RESULTS = r
        total = np.float64(0.0)
        for c in range(N_CORES):
            total += np.asarray(r.results[c]["partial"], np.float64).sum()
        return np.float32(-total / float(E))
    except Exception:
        if _trace:
            raise
        return _np_fallback(label, log_theta, log_alpha, idx)
